# revision 5
# baseline (speedup 1.0000x reference)
"""Trainium2 Bass kernel for nn_ConcatHandshaking.

Computes out[b, p, :] = tanh(proj_i[b, ii[p], :] + proj_j[b, jj[p], :])
where proj_i = hidden @ W[:D], proj_j = hidden @ W[D:] + bias, and (ii, jj)
are the upper-triangular token pairs of a length-S sequence.

Sharding: data-parallel over batch. B=16 batches -> 2 per core on 8 cores.

Per-core pipeline (per batch):
  Stage A: PE-transpose hidden -> hiddenT; matmul hiddenT x W-halves -> proj_i,
           proj_j (S=128 partitions, H=768 free) in SBUF.
  Stage B: pair axis (P=8256) tiled into 65 chunks of 128 pairs. For each
           chunk, two accumulating PE matmuls with host-built 0/1 selector
           matrices compute selI.T @ proj_i + selJ.T @ proj_j in PSUM
           (a fused gather+add), then one ScalarE tanh writes PSUM -> SBUF,
           and chunks are staged in groups of 5 for ~2MB contiguous DMAs.
"""

import sys

if "/opt/trn_rl_repo" not in sys.path:
    sys.path.insert(0, "/opt/trn_rl_repo")

from contextlib import ExitStack

import numpy as np

import concourse.bass as bass
import concourse.mybir as mybir
import concourse.tile as tile
from concourse import bacc
from concourse.bass_utils import run_bass_kernel_spmd
from concourse.masks import make_identity

B, S, D, H = 16, 128, 768, 768
P = S * (S + 1) // 2  # 8256 upper-triangular pairs
NCORES = 8
BPC = B // NCORES  # batches per core
NT = (P + S - 1) // S  # 65 pair tiles of 128 pairs (last has 64 valid)
GRP = 5  # pair tiles per output staging group
NGRP = (NT + GRP - 1) // GRP  # 13 groups (last group has the ragged tile)

F32 = mybir.dt.float32
F32R = mybir.dt.float32r

# Stage-B matmul dtype: float32r runs the PE at 1 cycle/column (vs 4 for
# fp32); operands are 0/1 selectors and fp32 projections.
STAGEB_F32R = True


def _build_nc() -> bass.Bass:
    nc = bacc.Bacc(
        "TRN2", target_bir_lowering=False, debug=False, num_devices=NCORES
    )

    hidden = nc.declare_dram_parameter("hidden", [BPC, S, D], F32, isOutput=False)
    w = nc.declare_dram_parameter("w", [2 * D, H], F32, isOutput=False)
    bias_bc = nc.declare_dram_parameter("bias_bc", [S, H], F32, isOutput=False)
    sel_i = nc.declare_dram_parameter("sel_i", [S, NT, S], F32R, isOutput=False)
    sel_j = nc.declare_dram_parameter("sel_j", [S, NT, S], F32R, isOutput=False)
    out = nc.declare_dram_parameter("out", [BPC, P, H], F32, isOutput=True)

    with tile.TileContext(nc) as tc, ExitStack() as ctx:
        consts = ctx.enter_context(tc.tile_pool(name="consts", bufs=1))
        acts = ctx.enter_context(tc.tile_pool(name="acts", bufs=2))
        outs = ctx.enter_context(tc.tile_pool(name="outs", bufs=3))
        psum_pair = ctx.enter_context(
            tc.tile_pool(name="psum_pair", bufs=2, space="PSUM")
        )
        psum_proj = ctx.enter_context(
            tc.tile_pool(name="psum_proj", bufs=1, space="PSUM")
        )
        psum_tr = ctx.enter_context(tc.tile_pool(name="psum_tr", bufs=2, space="PSUM"))

        # ---- one-time constants -------------------------------------------
        ident = consts.tile([S, S], F32)
        make_identity(nc, ident)

        # W halves laid out for rhs: (k=128 d-partitions, 6 d-chunks, H)
        w1_sb = consts.tile([128, D // 128, H], F32)
        nc.sync.dma_start(w1_sb[:], w[0:D].rearrange("(c k) h -> k c h", k=128))
        w2_sb = consts.tile([128, D // 128, H], F32)
        nc.sync.dma_start(w2_sb[:], w[D : 2 * D].rearrange("(c k) h -> k c h", k=128))

        bias_sb = consts.tile([S, H], F32)
        nc.sync.dma_start(bias_sb[:], bias_bc[:])

        seli_mm = consts.tile([S, NT, S], F32R, name="seli_mm")
        nc.sync.dma_start(seli_mm[:], sel_i[:])
        selj_mm = consts.tile([S, NT, S], F32R, name="selj_mm")
        nc.sync.dma_start(selj_mm[:], sel_j[:])

        for bb in range(BPC):
            # ---- stage A: projections ------------------------------------
            hid = acts.tile([S, D], F32)
            nc.sync.dma_start(hid[:], hidden[bb])

            hid_t = acts.tile([128, D // 128, S], F32)
            for c in range(D // 128):
                tr_ps = psum_tr.tile([128, 128], F32)
                nc.tensor.transpose(tr_ps[:], hid[:, bass.ts(c, 128)], ident[:])
                nc.vector.tensor_copy(hid_t[:, c, :], tr_ps[:])

            proj_i = acts.tile([S, H], F32R if STAGEB_F32R else F32)
            proj_j = acts.tile([S, H], F32R if STAGEB_F32R else F32)
            for which, (w_sb, proj) in enumerate(
                [(w1_sb, proj_i), (w2_sb, proj_j)]
            ):
                pp = psum_proj.tile([128, 1024], F32, name=f"pp_{bb}_{which}", tag="pp")
                nchunks = D // 128
                for c in range(nchunks):
                    nc.tensor.matmul(
                        pp[:, 0:512],
                        lhsT=hid_t[:, c, :],
                        rhs=w_sb[:, c, 0:512],
                        start=(c == 0),
                        stop=(c == nchunks - 1),
                    )
                for c in range(nchunks):
                    nc.tensor.matmul(
                        pp[:, 512:768],
                        lhsT=hid_t[:, c, :],
                        rhs=w_sb[:, c, 512:768],
                        start=(c == 0),
                        stop=(c == nchunks - 1),
                    )
                if which == 0:
                    nc.vector.tensor_copy(proj[:], pp[:, 0:H])
                else:
                    nc.vector.tensor_add(proj[:], pp[:, 0:H], bias_sb[:])

            pi_mm, pj_mm = proj_i, proj_j

            # ---- stage B: pair tiles -------------------------------------
            for g in range(NGRP):
                t0 = g * GRP
                ntile = min(GRP, NT - t0)
                og = outs.tile([128, GRP, H], F32, name=f"og_{bb}_{g}", tag="og")
                for tt in range(ntile):
                    t = t0 + tt
                    pq = psum_pair.tile([128, 1024], F32, name=f"pq_{bb}_{g}_{tt}", tag="pq")
                    for lo, n in ((0, 512), (512, 256)):
                        nc.tensor.matmul(
                            pq[:, lo : lo + n],
                            lhsT=seli_mm[:, t, :],
                            rhs=pi_mm[:, lo : lo + n],
                            start=True,
                            stop=False,
                        )
                        nc.tensor.matmul(
                            pq[:, lo : lo + n],
                            lhsT=selj_mm[:, t, :],
                            rhs=pj_mm[:, lo : lo + n],
                            start=False,
                            stop=True,
                        )
                    nc.scalar.activation(
                        og[:, tt, :], pq[:, 0:H], mybir.ActivationFunctionType.Tanh
                    )
                # DMA the group out. Full tiles are contiguous rows of out;
                # the final tile of the last group only has 64 valid pairs.
                r0 = t0 * S
                nfull = ntile if t0 + ntile < NT else ntile - 1
                if nfull > 0:
                    nc.sync.dma_start(
                        out[bb, r0 : r0 + nfull * S, :].rearrange(
                            "(g p) h -> p g h", p=S
                        ),
                        og[:, 0:nfull, :],
                    )
                if t0 + ntile == NT:
                    tail = P - (NT - 1) * S  # 64
                    nc.sync.dma_start(
                        out[bb, (NT - 1) * S : P, :],
                        og[0:tail, ntile - 1, :],
                    )

    nc.compile()
    return nc


_NC_CACHE: bass.Bass | None = None


def _get_nc() -> bass.Bass:
    global _NC_CACHE
    if _NC_CACHE is None:
        _NC_CACHE = _build_nc()
    return _NC_CACHE


def _selectors() -> tuple[np.ndarray, np.ndarray]:
    ii, jj = np.triu_indices(S)
    sel_i = np.zeros((S, NT, S), dtype=np.float32)
    sel_j = np.zeros((S, NT, S), dtype=np.float32)
    for p in range(P):
        t, m = divmod(p, S)
        sel_i[ii[p], t, m] = 1.0
        sel_j[jj[p], t, m] = 1.0
    return sel_i, sel_j


def kernel(hidden: np.ndarray, W: np.ndarray, b: np.ndarray) -> np.ndarray:
    hidden = np.ascontiguousarray(hidden, dtype=np.float32)
    W = np.ascontiguousarray(W, dtype=np.float32)
    b = np.ascontiguousarray(b, dtype=np.float32)

    sel_i, sel_j = _selectors()
    bias_bc = np.ascontiguousarray(np.broadcast_to(b, (S, H)), dtype=np.float32)

    nc = _get_nc()
    in_maps = []
    for c in range(NCORES):
        in_maps.append(
            {
                "hidden": hidden[c * BPC : (c + 1) * BPC],
                "w": W,
                "bias_bc": bias_bc,
                "sel_i": sel_i,
                "sel_j": sel_j,
            }
        )
    res = run_bass_kernel_spmd(nc, in_maps, list(range(NCORES)))
    return np.concatenate([res.results[c]["out"] for c in range(NCORES)], axis=0)


# revision 6
# speedup vs baseline: 1.4154x; 1.4154x over previous
"""Trainium2 Bass kernel for nn_ConcatHandshaking.

Computes out[b, p, :] = tanh(proj_i[b, ii[p], :] + proj_j[b, jj[p], :])
where proj_i = hidden @ W[:D], proj_j = hidden @ W[D:] + bias, and (ii, jj)
are the upper-triangular token pairs of a length-S sequence.

Sharding: data-parallel over batch. B=16 batches -> 2 per core on 8 cores.

Per-core pipeline (per batch):
  Stage A: PE-transpose hidden -> hiddenT; matmul hiddenT x W-halves -> proj_i,
           proj_j (S=128 partitions, H=768 free) in SBUF.
  Stage B: pair axis (P=8256) tiled into 65 chunks of 128 pairs. For each
           chunk, two accumulating PE matmuls with host-built 0/1 selector
           matrices compute selI.T @ proj_i + selJ.T @ proj_j in PSUM
           (a fused gather+add), then one ScalarE tanh writes PSUM -> SBUF,
           and chunks are staged in groups of 5 for ~2MB contiguous DMAs.
"""

import sys

if "/opt/trn_rl_repo" not in sys.path:
    sys.path.insert(0, "/opt/trn_rl_repo")

from contextlib import ExitStack

import numpy as np

import concourse.bass as bass
import concourse.mybir as mybir
import concourse.tile as tile
from concourse import bacc
from concourse.bass_utils import run_bass_kernel_spmd
from concourse.masks import make_identity

B, S, D, H = 16, 128, 768, 768
P = S * (S + 1) // 2  # 8256 upper-triangular pairs
NCORES = 8
BPC = B // NCORES  # batches per core
NT = (P + S - 1) // S  # 65 pair tiles of 128 pairs (last has 64 valid)
GRP = 5  # pair tiles per output staging group
NGRP = (NT + GRP - 1) // GRP  # 13 groups (last group has the ragged tile)

F32 = mybir.dt.float32
F32R = mybir.dt.float32r

# Stage-B matmul dtype: float32r runs the PE at 1 cycle/column (vs 4 for
# fp32); operands are 0/1 selectors and fp32 projections.
STAGEB_F32R = True


def _build_nc(repeat: int = 1) -> bass.Bass:
    nc = bacc.Bacc(
        "TRN2", target_bir_lowering=False, debug=False, num_devices=NCORES
    )

    hidden = nc.declare_dram_parameter("hidden", [BPC, S, D], F32, isOutput=False)
    w = nc.declare_dram_parameter("w", [2 * D, H], F32, isOutput=False)
    bias_bc = nc.declare_dram_parameter("bias_bc", [S, H], F32, isOutput=False)
    sel_i = nc.declare_dram_parameter("sel_i", [S, NT, S], F32R, isOutput=False)
    sel_j = nc.declare_dram_parameter("sel_j", [S, NT, S], F32R, isOutput=False)
    out = nc.declare_dram_parameter("out", [BPC, P, H], F32, isOutput=True)

    with tile.TileContext(nc) as tc, ExitStack() as ctx:
        consts = ctx.enter_context(tc.tile_pool(name="consts", bufs=1))
        acts = ctx.enter_context(tc.tile_pool(name="acts", bufs=2))
        outs = ctx.enter_context(tc.tile_pool(name="outs", bufs=3))
        psum_pair = ctx.enter_context(
            tc.tile_pool(name="psum_pair", bufs=2, space="PSUM")
        )
        psum_proj = ctx.enter_context(
            tc.tile_pool(name="psum_proj", bufs=1, space="PSUM")
        )
        psum_tr = ctx.enter_context(tc.tile_pool(name="psum_tr", bufs=2, space="PSUM"))

        # ---- one-time constants -------------------------------------------
        ident = consts.tile([S, S], F32)
        make_identity(nc, ident)

        # W halves laid out for rhs: (k=128 d-partitions, 6 d-chunks, H)
        w1_sb = consts.tile([128, D // 128, H], F32)
        nc.sync.dma_start(w1_sb[:], w[0:D].rearrange("(c k) h -> k c h", k=128))
        w2_sb = consts.tile([128, D // 128, H], F32)
        nc.sync.dma_start(w2_sb[:], w[D : 2 * D].rearrange("(c k) h -> k c h", k=128))

        bias_sb = consts.tile([S, H], F32)
        nc.sync.dma_start(bias_sb[:], bias_bc[:])

        seli_mm = consts.tile([S, NT, S], F32R, name="seli_mm")
        nc.sync.dma_start(seli_mm[:], sel_i[:])
        selj_mm = consts.tile([S, NT, S], F32R, name="selj_mm")
        nc.sync.dma_start(selj_mm[:], sel_j[:])

        for bb_rep in range(BPC * repeat):
            bb = bb_rep % BPC
            # ---- stage A: projections ------------------------------------
            hid = acts.tile([S, D], F32)
            nc.sync.dma_start(hid[:], hidden[bb])

            hid_t = acts.tile([128, D // 128, S], F32)
            for c in range(D // 128):
                tr_ps = psum_tr.tile([128, 128], F32)
                nc.tensor.transpose(tr_ps[:], hid[:, bass.ts(c, 128)], ident[:])
                nc.vector.tensor_copy(hid_t[:, c, :], tr_ps[:])

            proj_i = acts.tile([S, H], F32R if STAGEB_F32R else F32)
            proj_j = acts.tile([S, H], F32R if STAGEB_F32R else F32)
            for which, (w_sb, proj) in enumerate(
                [(w1_sb, proj_i), (w2_sb, proj_j)]
            ):
                pp = psum_proj.tile([128, 1024], F32, name=f"pp_{bb}_{which}", tag="pp")
                nchunks = D // 128
                for c in range(nchunks):
                    nc.tensor.matmul(
                        pp[:, 0:512],
                        lhsT=hid_t[:, c, :],
                        rhs=w_sb[:, c, 0:512],
                        start=(c == 0),
                        stop=(c == nchunks - 1),
                    )
                for c in range(nchunks):
                    nc.tensor.matmul(
                        pp[:, 512:768],
                        lhsT=hid_t[:, c, :],
                        rhs=w_sb[:, c, 512:768],
                        start=(c == 0),
                        stop=(c == nchunks - 1),
                    )
                if which == 0:
                    nc.vector.tensor_copy(proj[:], pp[:, 0:H])
                else:
                    nc.vector.tensor_add(proj[:], pp[:, 0:H], bias_sb[:])

            pi_mm, pj_mm = proj_i, proj_j

            # ---- stage B: pair tiles -------------------------------------
            for g in range(NGRP):
                t0 = g * GRP
                ntile = min(GRP, NT - t0)
                og = outs.tile([128, GRP, H], F32, name=f"og_{bb}_{g}", tag="og")
                for tt in range(ntile):
                    t = t0 + tt
                    pq = psum_pair.tile([128, 1024], F32, name=f"pq_{bb}_{g}_{tt}", tag="pq")
                    for lo, n in ((0, 512), (512, 256)):
                        nc.tensor.matmul(
                            pq[:, lo : lo + n],
                            lhsT=seli_mm[:, t, :],
                            rhs=pi_mm[:, lo : lo + n],
                            start=True,
                            stop=False,
                        )
                        nc.tensor.matmul(
                            pq[:, lo : lo + n],
                            lhsT=selj_mm[:, t, :],
                            rhs=pj_mm[:, lo : lo + n],
                            start=False,
                            stop=True,
                        )
                    nc.scalar.activation(
                        og[:, tt, :], pq[:, 0:H], mybir.ActivationFunctionType.Tanh
                    )
                # DMA the group out. Full tiles are contiguous rows of out;
                # the final tile of the last group only has 64 valid pairs.
                r0 = t0 * S
                nfull = ntile if t0 + ntile < NT else ntile - 1
                if nfull > 0:
                    nc.sync.dma_start(
                        out[bb, r0 : r0 + nfull * S, :].rearrange(
                            "(g p) h -> p g h", p=S
                        ),
                        og[:, 0:nfull, :],
                    )
                if t0 + ntile == NT:
                    tail = P - (NT - 1) * S  # 64
                    nc.sync.dma_start(
                        out[bb, (NT - 1) * S : P, :],
                        og[0:tail, ntile - 1, :],
                    )

    nc.compile()
    return nc


_NC_CACHE: dict[int, bass.Bass] = {}


def _get_nc(repeat: int = 1) -> bass.Bass:
    if repeat not in _NC_CACHE:
        _NC_CACHE[repeat] = _build_nc(repeat)
    return _NC_CACHE[repeat]


def _selectors() -> tuple[np.ndarray, np.ndarray]:
    ii, jj = np.triu_indices(S)
    sel_i = np.zeros((S, NT, S), dtype=np.float32)
    sel_j = np.zeros((S, NT, S), dtype=np.float32)
    for p in range(P):
        t, m = divmod(p, S)
        sel_i[ii[p], t, m] = 1.0
        sel_j[jj[p], t, m] = 1.0
    return sel_i, sel_j


def kernel(hidden: np.ndarray, W: np.ndarray, b: np.ndarray) -> np.ndarray:
    hidden = np.ascontiguousarray(hidden, dtype=np.float32)
    W = np.ascontiguousarray(W, dtype=np.float32)
    b = np.ascontiguousarray(b, dtype=np.float32)

    sel_i, sel_j = _selectors()
    bias_bc = np.ascontiguousarray(np.broadcast_to(b, (S, H)), dtype=np.float32)

    nc = _get_nc()
    in_maps = []
    for c in range(NCORES):
        in_maps.append(
            {
                "hidden": hidden[c * BPC : (c + 1) * BPC],
                "w": W,
                "bias_bc": bias_bc,
                "sel_i": sel_i,
                "sel_j": sel_j,
            }
        )
    res = run_bass_kernel_spmd(nc, in_maps, list(range(NCORES)))
    return np.concatenate([res.results[c]["out"] for c in range(NCORES)], axis=0)
